# revision 25
# baseline (speedup 1.0000x reference)
"""Trainium2 Bass kernel for nn_CoOccurrenceGraph.

Computation (full problem: B=64, C=512, D=1024):
    ew  = edge_weights(co_occurrence, class_counts, context_embeddings)  # [C,C]
    x_t = ew @ x[b]                          # per batch
    gate = sigmoid(sum(x*x_t, -1)/sqrt(D))   # [B,C,1]
    out  = x*(1-gate) + x_t*gate

Data-parallel over batch across 8 NeuronCores (8 batches/core).

Final design (131.5us baseline -> ~81us measured):
  * The [C,C] edge-weight build is O(C^2) scalar work -- 0.01% of the
    FLOPs -- precomputed on the host in float64 (the baseline already
    precomputed all per-class O(C) vectors plus sum(x^2) on the host).
    The device gets AT = (ew - I)^T as bf16 and does the heavy part:
    per batch  d = (ew-I) @ x,  gate, combine.
  * PE streams 256 bf16 matmuls back-to-back (216ns cadence, warm):
    per output tile [128,1024]: 4 k-chunks x 2 n-halves into a 2-bank
    PSUM tile, 4-deep PSUM pipelining.
  * Per-tile epilogue, balanced so every engine is under the 1.73us
    PE tile cadence:
      DVE:  gs = rowsum(x*d/sqrt(D))   (STT from PSUM, ~1.25us)
      ACT:  gate = sigmoid(gs + ssb);  d_g = gate*d (PSUM->SBUF bf16
            copy with per-partition scale -- also frees the PSUM pair)
      DVE/GpSimd (alternating): out = d_g + x  (bf16 TT)
    The add for tile t is emitted after tile t+1's gs so the DVE FIFO
    never head-of-line blocks on the ACT round trip.
  * All HBM traffic is bf16: first batch + weights arrive as small
    chunked DMAs (fast start), the rest as 1MB transfers; y stores are
    1MB per batch (last batch per-tile). Host casts y back to f32.
"""

import numpy as np

import concourse.bass as bass
import concourse.bacc as bacc
import concourse.mybir as mybir
import concourse.tile as tile
from concourse.bass_utils import run_bass_kernel_spmd

F32 = mybir.dt.float32
BF16 = mybir.dt.bfloat16
OP = mybir.AluOpType
AF = mybir.ActivationFunctionType

B, C, D = 64, 512, 1024
P = 128
NCORES = 8
BPC = B // NCORES          # batches per core
CT = C // P                # 4 chunks of 128 rows
SMOOTH = 0.01
THRESH = 0.5
SCALING = 5.0
INV_SQRT_D = 1.0 / float(np.sqrt(D))

_CACHE = {}


def _build_module():
    nc = bacc.Bacc("TRN2", target_bir_lowering=False, debug=False,
                   num_devices=NCORES)
    dt = nc.dram_tensor
    xh_d = dt("xh", [BPC, P, CT * D], BF16, kind="ExternalInput").ap()
    at_d = dt("at", [C, C], BF16, kind="ExternalInput").ap()
    ssb_d = dt("ssb", [P, BPC * CT], F32, kind="ExternalInput").ap()
    y_d = dt("y", [BPC, P, CT * D], BF16, kind="ExternalOutput").ap()

    with tile.TileContext(nc) as tc:
        _body(nc, tc, xh_d, at_d, ssb_d, y_d)
    if not nc.is_finalized():
        nc.finalize()
    return nc


def _body(nc, tc, xh_d, at_d, ssb_d, y_d):
    from contextlib import ExitStack
    with ExitStack() as ctx:
        persist = ctx.enter_context(tc.tile_pool(name="persist", bufs=1))
        dgp = ctx.enter_context(tc.tile_pool(name="dg", bufs=6))
        g2p = ctx.enter_context(tc.tile_pool(name="g2", bufs=3))
        tiny = ctx.enter_context(tc.tile_pool(name="tiny", bufs=8))
        obp = ctx.enter_context(tc.tile_pool(name="ob", bufs=3))
        psB = ctx.enter_context(tc.tile_pool(name="ps", bufs=4, space="PSUM"))

        # warm the ACT sigmoid table set during the DMA prologue so the
        # ~2.6us ACT_TABLE_LOAD doesn't stall the first in-stream sigmoid
        warm = tiny.tile([P, 1], F32, tag="warm")
        nc.vector.memset(warm[:], 0.0)
        nc.scalar.activation(warm[:], warm[:], AF.Sigmoid)

        # warm the PE HAM clock gate during the ~4us x0/at load window:
        # ~8 dummy matmuls on zeroed tiles give the 4096-cycle activity
        # window enough busy time that the real stream starts at 2.4 GHz
        # (without this, a bad HAM phase costs 2-5us of throttled stream)
        wz = persist.tile([P, P], BF16, tag="wz")
        nc.vector.memset(wz[:], 0.0)
        xz = persist.tile([P, 512], BF16, tag="xz")
        nc.vector.memset(xz[:], 0.0)
        warm_ps = psB.tile([P, D], F32, tag="d")
        for i in range(8):
            nc.tensor.matmul(warm_ps[:, 0:512], wz[:], xz[:],
                             start=(i == 0), stop=(i == 7))

        # ---- weights + first batch interleaved in 128K/256K chunks so the
        # ---- first matmul can start as soon as chunk 0 of each lands
        at_t = persist.tile([P, CT * C], BF16, tag="at")
        x0 = persist.tile([P, CT * D], BF16, tag="x0")
        for k in range(CT):
            nc.sync.dma_start(at_t[:, bass.ts(k, C)], at_d[bass.ts(k, P), :])
            nc.sync.dma_start(x0[:, bass.ts(k, D)], xh_d[0][:, bass.ts(k, D)])
        ssb_t = persist.tile([P, BPC * CT], F32, tag="ssb")
        nc.sync.dma_start(ssb_t[:], ssb_d[:])

        xt = [x0]
        for b in range(1, BPC):
            xb = persist.tile([P, CT * D], BF16, tag=f"x{b}")
            if b == 1:
                for k in range(CT):
                    nc.sync.dma_start(xb[:, bass.ts(k, D)],
                                      xh_d[b][:, bass.ts(k, D)])
            else:
                nc.sync.dma_start(xb[:], xh_d[b])
            xt.append(xb)

        pending_add = []        # (d_g, xm, out_slice, engine, store or None)

        def flush_one():
            d_g, xm, o_sl, eng, store = pending_add.pop(0)
            eng(o_sl, d_g[:], xm, OP.add)
            if store is not None:
                store()

        tile_no = 0
        for b in range(BPC):
            o_all = obp.tile([P, CT * D], BF16, tag="o")
            for m in range(CT):
                d_ps = psB.tile([P, D], F32, tag="d")
                for k in range(CT):
                    lhsT = at_t[:, k * C + m * P: k * C + (m + 1) * P]
                    for n in range(2):
                        nc.tensor.matmul(
                            d_ps[:, bass.ts(n, 512)], lhsT,
                            xt[b][:, k * D + n * 512: k * D + (n + 1) * 512],
                            start=(k == 0), stop=(k == CT - 1))
                xm = xt[b][:, bass.ts(m, D)]
                # gs = sum(x*d)/sqrt(D), straight from PSUM
                gs = tiny.tile([P, 1], F32, tag="gs")
                g2 = g2p.tile([P, D], BF16, tag="g2")
                nc.vector.scalar_tensor_tensor(
                    g2[:], xm, INV_SQRT_D, d_ps[:],
                    OP.mult, OP.mult, accum_out=gs[:])
                # gate = sigmoid(gs + sum(x^2)/sqrt(D))
                gate = tiny.tile([P, 1], F32, tag="gate")
                nc.scalar.activation(gate[:], gs[:], AF.Sigmoid,
                                     bias=ssb_t[:, b * CT + m:
                                                b * CT + m + 1])
                if b == BPC - 1 and m == CT - 1:
                    # tail tile: single STT combine straight from PSUM skips
                    # the ACT evac (the ACT queue is the drain bottleneck:
                    # ~4.7us of serial evac/sigmoid after the last matmul)
                    while pending_add:
                        flush_one()
                    nc.vector.scalar_tensor_tensor(
                        o_all[:, bass.ts(m, D)], d_ps[:], gate[:], xm,
                        OP.mult, OP.add)
                    nc.sync.dma_start(y_d[b][:, bass.ts(m, D)],
                                      o_all[:, bass.ts(m, D)])
                    tile_no += 1
                    continue
                # d_g = gate*d via ACT scale-copy (frees the PSUM pair)
                d_g = dgp.tile([P, D], BF16, tag="dg")
                nc.scalar.activation(d_g[:], d_ps[:], AF.Copy,
                                     scale=gate[:])
                # out = d_g + x, alternating GpSimd/DVE, delayed one tile;
                # tile 29's add also to GpSimd so the DVE queue is clear of
                # adds when the tail tile's gs starts
                eng = (nc.gpsimd.tensor_tensor
                       if (tile_no % 2 == 0 or tile_no == BPC * CT - 3)
                       else nc.vector.tensor_tensor)
                store = None
                if m == CT - 1 and b < BPC - 1:
                    def store(b=b, o_all=o_all):
                        nc.sync.dma_start(y_d[b], o_all[:])
                elif b == BPC - 1:
                    # per-tile stores for the last batch keep the tail short
                    def store(b=b, o_all=o_all, m=m):
                        nc.sync.dma_start(y_d[b][:, bass.ts(m, D)],
                                          o_all[:, bass.ts(m, D)])
                pending_add.append(
                    (d_g, xm, o_all[:, bass.ts(m, D)], eng, store))
                if len(pending_add) > 1:
                    flush_one()
                tile_no += 1
        while pending_add:
            flush_one()


def _edge_weights_host(co, cnt, emb):
    """Exact reference edge-weight build, in float64."""
    s = SMOOTH
    eye = np.eye(C)
    off = 1.0 - eye
    avg = cnt.mean()
    denom = np.sqrt((cnt[:, None] + s) * (cnt[None, :] + s))
    norm_co = (co + s) / denom
    nemb = emb / np.linalg.norm(emb, axis=1, keepdims=True)
    sim = nemb @ nemb.T
    aff = sim / (1.0 + np.exp(-(sim - THRESH) * 10.0))
    minc = np.minimum(cnt[:, None], cnt[None, :])
    maxc = np.maximum(cnt[:, None], cnt[None, :])
    bal = np.where((minc > s) & (maxc > s),
                   np.log1p(maxc / avg) * (minc / maxc), s)
    conf = 2.0 / (1.0 + np.exp(-co / SCALING)) - 1.0
    ew = norm_co * aff * bal * conf * off
    m = ew * 5.0
    e = np.exp(m - m.max(axis=1, keepdims=True))
    sm = e / e.sum(axis=1, keepdims=True)
    return sm * 0.9 + eye * 0.1


LAST_RESULTS = None


def kernel(x, co_occurrence, class_counts, context_embeddings, _trace=False):
    global LAST_RESULTS
    if "nc" not in _CACHE:
        _CACHE["nc"] = _build_module()
    nc = _CACHE["nc"]

    import ml_dtypes
    co = np.asarray(co_occurrence, dtype=np.float64)
    cnt = np.asarray(class_counts, dtype=np.float64)
    emb = np.asarray(context_embeddings, dtype=np.float64)

    ew = _edge_weights_host(co, cnt, emb)
    at = np.ascontiguousarray(
        (ew - np.eye(C)).T.astype(ml_dtypes.bfloat16))

    x_bf = np.asarray(x, dtype=np.float32).astype(ml_dtypes.bfloat16)
    xs32 = x_bf.astype(np.float32)
    ss = np.einsum('bcd,bcd->bc', xs32, xs32) * INV_SQRT_D   # [B, C] f32

    ins = {"at": at}
    in_maps = []
    for c in range(NCORES):
        m = dict(ins)
        xc = x_bf[c * BPC:(c + 1) * BPC]
        m["xh"] = np.ascontiguousarray(
            xc.reshape(BPC, CT, P, D).transpose(0, 2, 1, 3)
              .reshape(BPC, P, CT * D))
        sc = ss[c * BPC:(c + 1) * BPC]                        # [BPC, C]
        m["ssb"] = np.ascontiguousarray(
            sc.reshape(BPC, CT, P).transpose(2, 0, 1)
              .reshape(P, BPC * CT).astype(np.float32))
        in_maps.append(m)
    res = run_bass_kernel_spmd(nc, in_maps, list(range(NCORES)), trace=_trace)
    LAST_RESULTS = res

    outs = []
    for r in res.results:
        yc = np.asarray(r["y"]).astype(np.float32)            # [BPC, P, CT*D]
        outs.append(yc.reshape(BPC, P, CT, D).transpose(0, 2, 1, 3)
                      .reshape(BPC, C, D))
    return np.concatenate(outs, axis=0)


# revision 26
# speedup vs baseline: 1.0293x; 1.0293x over previous
"""Trainium2 Bass kernel for nn_CoOccurrenceGraph.

Computation (full problem: B=64, C=512, D=1024):
    ew  = edge_weights(co_occurrence, class_counts, context_embeddings)  # [C,C]
    x_t = ew @ x[b]                          # per batch
    gate = sigmoid(sum(x*x_t, -1)/sqrt(D))   # [B,C,1]
    out  = x*(1-gate) + x_t*gate

Data-parallel over batch across 8 NeuronCores (8 batches/core).

Final design (131.5us baseline -> ~81us measured):
  * The [C,C] edge-weight build is O(C^2) scalar work -- 0.01% of the
    FLOPs -- precomputed on the host in float64 (the baseline already
    precomputed all per-class O(C) vectors plus sum(x^2) on the host).
    The device gets AT = (ew - I)^T as bf16 and does the heavy part:
    per batch  d = (ew-I) @ x,  gate, combine.
  * PE streams 256 bf16 matmuls back-to-back (216ns cadence, warm):
    per output tile [128,1024]: 4 k-chunks x 2 n-halves into a 2-bank
    PSUM tile, 4-deep PSUM pipelining.
  * Per-tile epilogue, balanced so every engine is under the 1.73us
    PE tile cadence:
      DVE:  gs = rowsum(x*d/sqrt(D))   (STT from PSUM, ~1.25us)
      ACT:  gate = sigmoid(gs + ssb);  d_g = gate*d (PSUM->SBUF bf16
            copy with per-partition scale -- also frees the PSUM pair)
      DVE/GpSimd (alternating): out = d_g + x  (bf16 TT)
    The add for tile t is emitted after tile t+1's gs so the DVE FIFO
    never head-of-line blocks on the ACT round trip.
  * All HBM traffic is bf16: first batch + weights arrive as small
    chunked DMAs (fast start), the rest as 1MB transfers; y stores are
    1MB per batch (last batch per-tile). Host casts y back to f32.
"""

import numpy as np

import concourse.bass as bass
import concourse.bacc as bacc
import concourse.mybir as mybir
import concourse.tile as tile
from concourse.bass_utils import run_bass_kernel_spmd

F32 = mybir.dt.float32
BF16 = mybir.dt.bfloat16
OP = mybir.AluOpType
AF = mybir.ActivationFunctionType

B, C, D = 64, 512, 1024
P = 128
NCORES = 8
BPC = B // NCORES          # batches per core
CT = C // P                # 4 chunks of 128 rows
SMOOTH = 0.01
THRESH = 0.5
SCALING = 5.0
INV_SQRT_D = 1.0 / float(np.sqrt(D))

_CACHE = {}


def _build_module():
    nc = bacc.Bacc("TRN2", target_bir_lowering=False, debug=False,
                   num_devices=NCORES)
    dt = nc.dram_tensor
    xh_d = dt("xh", [BPC, P, CT * D], BF16, kind="ExternalInput").ap()
    at_d = dt("at", [C, C], BF16, kind="ExternalInput").ap()
    ssb_d = dt("ssb", [P, BPC * CT], F32, kind="ExternalInput").ap()
    y_d = dt("y", [BPC, P, CT * D], BF16, kind="ExternalOutput").ap()

    with tile.TileContext(nc) as tc:
        _body(nc, tc, xh_d, at_d, ssb_d, y_d)
    if not nc.is_finalized():
        nc.finalize()
    return nc


def _body(nc, tc, xh_d, at_d, ssb_d, y_d):
    from contextlib import ExitStack
    with ExitStack() as ctx:
        persist = ctx.enter_context(tc.tile_pool(name="persist", bufs=1))
        dgp = ctx.enter_context(tc.tile_pool(name="dg", bufs=6))
        g2p = ctx.enter_context(tc.tile_pool(name="g2", bufs=3))
        tiny = ctx.enter_context(tc.tile_pool(name="tiny", bufs=8))
        obp = ctx.enter_context(tc.tile_pool(name="ob", bufs=3))
        psB = ctx.enter_context(tc.tile_pool(name="ps", bufs=4, space="PSUM"))

        # warm the ACT sigmoid table set during the DMA prologue so the
        # ~2.6us ACT_TABLE_LOAD doesn't stall the first in-stream sigmoid
        warm = tiny.tile([P, 1], F32, tag="warm")
        nc.vector.memset(warm[:], 0.0)
        nc.scalar.activation(warm[:], warm[:], AF.Sigmoid)

        # warm the PE HAM clock gate during the ~4us x0/at load window:
        # ~8 dummy matmuls on zeroed tiles give the 4096-cycle activity
        # window enough busy time that the real stream starts at 2.4 GHz
        # (without this, a bad HAM phase costs 2-5us of throttled stream)
        wz = persist.tile([P, P], BF16, tag="wz")
        nc.vector.memset(wz[:], 0.0)
        xz = persist.tile([P, 512], BF16, tag="xz")
        nc.vector.memset(xz[:], 0.0)
        warm_ps = psB.tile([P, D], F32, tag="d")
        for i in range(8):
            nc.tensor.matmul(warm_ps[:, 0:512], wz[:], xz[:],
                             start=(i == 0), stop=(i == 7))

        # ---- weights + first batch interleaved in 128K/256K chunks so the
        # ---- first matmul can start as soon as chunk 0 of each lands
        at_t = persist.tile([P, CT * C], BF16, tag="at")
        x0 = persist.tile([P, CT * D], BF16, tag="x0")
        for k in range(CT):
            nc.sync.dma_start(at_t[:, bass.ts(k, C)], at_d[bass.ts(k, P), :])
            nc.sync.dma_start(x0[:, bass.ts(k, D)], xh_d[0][:, bass.ts(k, D)])
        ssb_t = persist.tile([P, BPC * CT], F32, tag="ssb")
        nc.sync.dma_start(ssb_t[:], ssb_d[:])

        xt = [x0]
        for b in range(1, BPC):
            xb = persist.tile([P, CT * D], BF16, tag=f"x{b}")
            if b == 1:
                for k in range(CT):
                    nc.sync.dma_start(xb[:, bass.ts(k, D)],
                                      xh_d[b][:, bass.ts(k, D)])
            else:
                nc.sync.dma_start(xb[:], xh_d[b])
            xt.append(xb)

        pending_add = []        # (d_g, xm, out_slice, engine, store or None)

        def flush_one():
            d_g, xm, o_sl, eng, store = pending_add.pop(0)
            eng(o_sl, d_g[:], xm, OP.add)
            if store is not None:
                store()

        tile_no = 0
        for b in range(BPC):
            o_all = obp.tile([P, CT * D], BF16, tag="o")
            for m in range(CT):
                d_ps = psB.tile([P, D], F32, tag="d")
                for k in range(CT):
                    lhsT = at_t[:, k * C + m * P: k * C + (m + 1) * P]
                    for n in range(2):
                        nc.tensor.matmul(
                            d_ps[:, bass.ts(n, 512)], lhsT,
                            xt[b][:, k * D + n * 512: k * D + (n + 1) * 512],
                            start=(k == 0), stop=(k == CT - 1))
                xm = xt[b][:, bass.ts(m, D)]
                # gs = sum(x*d)/sqrt(D), straight from PSUM
                gs = tiny.tile([P, 1], F32, tag="gs")
                g2 = g2p.tile([P, D], BF16, tag="g2")
                nc.vector.scalar_tensor_tensor(
                    g2[:], xm, INV_SQRT_D, d_ps[:],
                    OP.mult, OP.mult, accum_out=gs[:])
                # gate = sigmoid(gs + sum(x^2)/sqrt(D))
                gate = tiny.tile([P, 1], F32, tag="gate")
                nc.scalar.activation(gate[:], gs[:], AF.Sigmoid,
                                     bias=ssb_t[:, b * CT + m:
                                                b * CT + m + 1])
                # d_g = gate*d via ACT scale-copy (frees the PSUM pair)
                d_g = dgp.tile([P, D], BF16, tag="dg")
                nc.scalar.activation(d_g[:], d_ps[:], AF.Copy,
                                     scale=gate[:])
                # out = d_g + x, alternating GpSimd/DVE, delayed one tile
                eng = (nc.gpsimd.tensor_tensor if tile_no % 2 == 0
                       else nc.vector.tensor_tensor)
                store = None
                if m == CT - 1 and b < BPC - 1:
                    def store(b=b, o_all=o_all):
                        nc.sync.dma_start(y_d[b], o_all[:])
                elif b == BPC - 1:
                    # per-tile stores for the last batch keep the tail short
                    def store(b=b, o_all=o_all, m=m):
                        nc.sync.dma_start(y_d[b][:, bass.ts(m, D)],
                                          o_all[:, bass.ts(m, D)])
                pending_add.append(
                    (d_g, xm, o_all[:, bass.ts(m, D)], eng, store))
                if len(pending_add) > 1:
                    flush_one()
                tile_no += 1
        while pending_add:
            flush_one()


def _edge_weights_host(co, cnt, emb):
    """Exact reference edge-weight build, in float64."""
    s = SMOOTH
    eye = np.eye(C)
    off = 1.0 - eye
    avg = cnt.mean()
    denom = np.sqrt((cnt[:, None] + s) * (cnt[None, :] + s))
    norm_co = (co + s) / denom
    nemb = emb / np.linalg.norm(emb, axis=1, keepdims=True)
    sim = nemb @ nemb.T
    aff = sim / (1.0 + np.exp(-(sim - THRESH) * 10.0))
    minc = np.minimum(cnt[:, None], cnt[None, :])
    maxc = np.maximum(cnt[:, None], cnt[None, :])
    bal = np.where((minc > s) & (maxc > s),
                   np.log1p(maxc / avg) * (minc / maxc), s)
    conf = 2.0 / (1.0 + np.exp(-co / SCALING)) - 1.0
    ew = norm_co * aff * bal * conf * off
    m = ew * 5.0
    e = np.exp(m - m.max(axis=1, keepdims=True))
    sm = e / e.sum(axis=1, keepdims=True)
    return sm * 0.9 + eye * 0.1


LAST_RESULTS = None


def kernel(x, co_occurrence, class_counts, context_embeddings, _trace=False):
    global LAST_RESULTS
    if "nc" not in _CACHE:
        _CACHE["nc"] = _build_module()
    nc = _CACHE["nc"]

    import ml_dtypes
    co = np.asarray(co_occurrence, dtype=np.float64)
    cnt = np.asarray(class_counts, dtype=np.float64)
    emb = np.asarray(context_embeddings, dtype=np.float64)

    ew = _edge_weights_host(co, cnt, emb)
    at = np.ascontiguousarray(
        (ew - np.eye(C)).T.astype(ml_dtypes.bfloat16))

    x_bf = np.asarray(x, dtype=np.float32).astype(ml_dtypes.bfloat16)
    xs32 = x_bf.astype(np.float32)
    ss = np.einsum('bcd,bcd->bc', xs32, xs32) * INV_SQRT_D   # [B, C] f32

    ins = {"at": at}
    in_maps = []
    for c in range(NCORES):
        m = dict(ins)
        xc = x_bf[c * BPC:(c + 1) * BPC]
        m["xh"] = np.ascontiguousarray(
            xc.reshape(BPC, CT, P, D).transpose(0, 2, 1, 3)
              .reshape(BPC, P, CT * D))
        sc = ss[c * BPC:(c + 1) * BPC]                        # [BPC, C]
        m["ssb"] = np.ascontiguousarray(
            sc.reshape(BPC, CT, P).transpose(2, 0, 1)
              .reshape(P, BPC * CT).astype(np.float32))
        in_maps.append(m)
    res = run_bass_kernel_spmd(nc, in_maps, list(range(NCORES)), trace=_trace)
    LAST_RESULTS = res

    outs = []
    for r in res.results:
        yc = np.asarray(r["y"]).astype(np.float32)            # [BPC, P, CT*D]
        outs.append(yc.reshape(BPC, P, CT, D).transpose(0, 2, 1, 3)
                      .reshape(BPC, C, D))
    return np.concatenate(outs, axis=0)
